# revision 14
# baseline (speedup 1.0000x reference)
"""MixedExpertLayer Trainium2 kernel.

Dense data-parallel strategy: 16384 tokens sharded 8 ways (2048/core), expert
weights replicated. All 4 expert outputs are computed for every token and the
top-2 routing is applied as per-token coefficients c_e = sum_k w_k*[idx_k==e]
computed on device, so no data-dependent gather is needed.

Per-core layout: x is passed feature-major ([H, T+3] with a 3-column causal
halo) so gate/up matmuls contract H on partitions directly. A = silu(G)*U is
produced feature-major [I, T] and fed back as lhsT of the down matmul, which
yields token-major [tok, H] output. Conv experts run feature-major via
PE diagonal-matrix matmuls (4 taps accumulated in PSUM), then are transposed
into token-major with PE transpose. The final combine uses per-partition
(per-token) scalars on ACT, accumulating in SBUF bf16.

Compute dtype bf16 (PE 1 cycle/row), PSUM accumulation fp32.
"""

import numpy as np
import ml_dtypes

import concourse.bass as bass
import concourse.mybir as mybir
import concourse.tile as tile
from concourse.bass_utils import run_bass_kernel_spmd
from concourse.masks import make_identity

B, S, H, I, KTOP, KC = 4, 4096, 1024, 2048, 2, 4
NCORES = 8
T = (B * S) // NCORES          # 2048 tokens per core
TH = T + KC - 1                # 2051 cols with halo
TCH = 512                      # token chunk (matmul N / PSUM bank)
NCHUNK = T // TCH              # 4
NTS = TCH // 128               # 4 token subtiles per chunk
HK = H // 128                  # 8 h-chunks
IK = I // 128                  # 16 i-chunks
BF16 = mybir.dt.bfloat16
F32 = mybir.dt.float32
AF = mybir.ActivationFunctionType


def legalize_waits(nc):
    """This walrus build encodes exactly one sync-wait per instruction
    (single NEURON_ISA_TPB_EVENTS slot); Tile emits up to 3 plus a multi-wait
    tail Drain. Split extra waits onto wait-only EventSemaphore carriers
    inserted immediately before the instruction (same engine, same position,
    so no reordering and no deadlock risk)."""
    f = nc.m.functions[0]
    for blk in f.blocks:
        new = []
        for ins in list(blk.instructions):
            si = ins.sync_info
            if si is not None and si.on_wait and len(si.on_wait) > 1:
                best, order = {}, []
                for w in si.on_wait:
                    k = (w.sync_type, w.id, w.wait_mode)
                    if k not in best:
                        best[k] = w
                        order.append(k)
                    elif (w.wait_value or 0) > (best[k].wait_value or 0):
                        best[k] = w
                waits = [best[k] for k in order]
                for j, w in enumerate(waits[:-1]):
                    ev = mybir.InstEventSemaphore(
                        name=f"{ins.name}-lw{j}", engine=ins.engine, ins=[], outs=[],
                    )
                    ev.sync_info = mybir.SyncInfo(on_wait=[w], on_update=[])
                    new.append(ev)
                si.on_wait = [waits[-1]]
                ins.sync_info = si
            new.append(ins)
        blk.instructions = new
    return nc


def build_nc():
    nc = bass.Bass(num_devices=NCORES)
    xf = nc.dram_tensor("xf", [H, TH], BF16, kind="ExternalInput")
    wg = nc.dram_tensor("wg", [2, H, I], BF16, kind="ExternalInput")
    wu = nc.dram_tensor("wu", [2, H, I], BF16, kind="ExternalInput")
    wd = nc.dram_tensor("wd", [2, I, H], BF16, kind="ExternalInput")
    dgh = nc.dram_tensor("dgh", [2, HK, KC, 128, 128], BF16, kind="ExternalInput")
    idxp = nc.dram_tensor("idxp", [128, T // 128, KTOP], F32, kind="ExternalInput")
    nwp = nc.dram_tensor("nwp", [128, T // 128, KTOP], F32, kind="ExternalInput")
    out = nc.dram_tensor("out", [T, H], BF16, kind="ExternalOutput")

    xf_t = xf.rearrange("(o p) t -> p o t", p=128)        # [128, HK, TH]
    wg_t = [wg[e].rearrange("(o p) m -> p o m", p=128) for e in range(2)]
    wu_t = [wu[e].rearrange("(o p) m -> p o m", p=128) for e in range(2)]
    wd_t = [wd[e].rearrange("(o p) h -> p o h", p=128) for e in range(2)]

    with tile.TileContext(nc) as tc:
        with (
            tc.tile_pool(name="singles", bufs=1) as singles,
            tc.tile_pool(name="wpool", bufs=2) as wpool,
            tc.tile_pool(name="wdpool", bufs=18) as wdpool,
            tc.tile_pool(name="sf", bufs=18) as sfpool,
            tc.tile_pool(name="tmp", bufs=4) as tmp,
            tc.tile_pool(name="oa", bufs=6) as oapool,
            tc.tile_pool(name="diag", bufs=6) as diagpool,
            tc.tile_pool(name="ps", bufs=2, space="PSUM") as ps,
            tc.tile_pool(name="pd", bufs=2, space="PSUM") as pd,
        ):
            # ---- resident state ----
            xf_sb = singles.tile([128, HK, TH], BF16)
            nc.sync.dma_start(xf_sb, xf_t)

            ident = singles.tile([128, 128], BF16)
            make_identity(nc, ident)

            idxp_sb = singles.tile([128, T // 128, KTOP], F32)
            nc.sync.dma_start(idxp_sb, idxp[:])
            nwp_sb = singles.tile([128, T // 128, KTOP], F32)
            nc.sync.dma_start(nwp_sb, nwp[:])

            # routing coefficients c_tok[p, e, n] = sum_k nw[k]*[idx[k]==e]
            c_tok = singles.tile([128, 4, T // 128], F32)
            for e in range(4):
                eq = tmp.tile([128, T // 128, KTOP], F32, tag="eq")
                nc.vector.tensor_scalar(
                    out=eq, in0=idxp_sb, scalar1=float(e), scalar2=None,
                    op0=mybir.AluOpType.is_equal,
                )
                nc.vector.tensor_mul(eq, eq, nwp_sb)
                nc.vector.tensor_reduce(
                    out=c_tok[:, e, :], in_=eq, axis=mybir.AxisListType.X,
                    op=mybir.AluOpType.add,
                )

            # conv diag matrices diag(cw[e, hc*128: , j]), built host-side
            diag_sb = singles.tile([128, 2, HK, KC, 128], BF16)
            nc.sync.dma_start(diag_sb, dgh.rearrange("e hc j p m -> p e hc j m"))

            # A buffer: silu(G)*U feature-major, one expert at a time
            a_sb = singles.tile([128, IK, TCH], BF16)

            for c in range(NCHUNK):
                tok0 = c * TCH

                # ---- conv experts (2,3): feature-major, PE diag matmuls ----
                sts = {}
                for hc in range(HK):
                    for e in range(2):
                        psc = ps.tile([128, TCH], F32, tag="pg" if e == 0 else "pu")
                        for j in range(KC):
                            nc.tensor.matmul(
                                psc, diag_sb[:, e, hc, j, :],
                                xf_sb[:, hc, tok0 + j : tok0 + j + TCH],
                                start=(j == 0), stop=(j == KC - 1),
                            )
                        st = sfpool.tile([128, TCH], BF16, tag="sf")
                        nc.scalar.activation(out=st, in_=psc, func=AF.Silu)
                        sts[(e, hc)] = st

                # ---- MLP experts (0,1) ----
                for e in range(2):
                    # gate/up -> A  (feature-major [I, TCH])
                    for ig in range(4):
                        wgt = wpool.tile([128, HK, 512], BF16, tag="wg")
                        nc.sync.dma_start(wgt, wg_t[e][:, :, ig * 512 : (ig + 1) * 512])
                        wut = wpool.tile([128, HK, 512], BF16, tag="wu")
                        nc.sync.dma_start(wut, wu_t[e][:, :, ig * 512 : (ig + 1) * 512])
                        for ii in range(4):
                            i = ig * 4 + ii
                            psg = ps.tile([128, TCH], F32, tag="pg")
                            psu = ps.tile([128, TCH], F32, tag="pu")
                            for kc in range(HK):
                                nc.tensor.matmul(
                                    psg, wgt[:, kc, ii * 128 : (ii + 1) * 128],
                                    xf_sb[:, kc, 3 + tok0 : 3 + tok0 + TCH],
                                    start=(kc == 0), stop=(kc == HK - 1),
                                )
                            for kc in range(HK):
                                nc.tensor.matmul(
                                    psu, wut[:, kc, ii * 128 : (ii + 1) * 128],
                                    xf_sb[:, kc, 3 + tok0 : 3 + tok0 + TCH],
                                    start=(kc == 0), stop=(kc == HK - 1),
                                )
                            sg = tmp.tile([128, TCH], F32, tag="sg")
                            nc.scalar.activation(out=sg, in_=psg, func=AF.Silu)
                            nc.vector.tensor_mul(a_sb[:, i, :], sg, psu)

                    # down: token-major psum, post-scale by c_e
                    wds = []
                    for kc in range(IK):
                        wdt = wdpool.tile([128, H], BF16, tag="wd")
                        nc.sync.dma_start(wdt, wd_t[e][:, kc, :])
                        wds.append(wdt)
                    for ts_ in range(NTS):
                        psd = pd.tile([128, H], F32, tag="pd")
                        for kc in range(IK):
                            lhs = a_sb[:, kc, ts_ * 128 : (ts_ + 1) * 128]
                            nc.tensor.matmul(
                                psd[:, 0:512], lhs, wds[kc][:, 0:512],
                                start=(kc == 0), stop=(kc == IK - 1),
                            )
                            nc.tensor.matmul(
                                psd[:, 512:1024], lhs, wds[kc][:, 512:1024],
                                start=(kc == 0), stop=(kc == IK - 1),
                            )
                        n = c * NTS + ts_
                        if e == 0:
                            oa = oapool.tile([128, H], BF16, tag="oa")
                            sts[("oa", ts_)] = oa
                            nc.scalar.activation(
                                out=oa, in_=psd, func=AF.Copy,
                                scale=c_tok[:, 0, n : n + 1],
                            )
                        else:
                            tm = tmp.tile([128, H], BF16, tag="tm")
                            nc.scalar.activation(
                                out=tm, in_=psd, func=AF.Copy,
                                scale=c_tok[:, 1, n : n + 1],
                            )
                            oa = sts[("oa", ts_)]
                            nc.vector.tensor_add(oa, oa, tm)

                # ---- conv transpose to token-major + combine + store ----
                for ts_ in range(NTS):
                    n = c * NTS + ts_
                    oa = sts[("oa", ts_)]
                    for hg in range(2):
                        for e in range(2):
                            pst = ps.tile([128, TCH], BF16, tag="pg" if e == 0 else "pu")
                            for hh in range(4):
                                hc = hg * 4 + hh
                                nc.tensor.transpose(
                                    pst[:, hh * 128 : (hh + 1) * 128],
                                    sts[(e, hc)][:, ts_ * 128 : (ts_ + 1) * 128],
                                    ident,
                                )
                            tm = tmp.tile([128, TCH], BF16, tag="tmc")
                            nc.scalar.activation(
                                out=tm, in_=pst, func=AF.Copy,
                                scale=c_tok[:, 2 + e, n : n + 1],
                            )
                            nc.vector.tensor_add(
                                oa[:, hg * 512 : (hg + 1) * 512],
                                oa[:, hg * 512 : (hg + 1) * 512], tm,
                            )
                    nc.sync.dma_start(out[tok0 + ts_ * 128 : tok0 + (ts_ + 1) * 128, :], oa)
    return legalize_waits(nc)


def _bf16(a):
    return np.asarray(a).astype(ml_dtypes.bfloat16)


def build_in_maps(x, top_k_indices, norm_weights, mlp_gate, mlp_up, mlp_down, conv_w):
    xflat = np.asarray(x, dtype=np.float32).reshape(B * S, H)
    idxflat = np.asarray(top_k_indices).reshape(B * S, KTOP)
    nwflat = np.asarray(norm_weights, dtype=np.float32).reshape(B * S, KTOP)

    wg = _bf16(mlp_gate)
    wu = _bf16(mlp_up)
    wd = _bf16(mlp_down)
    # diag(cw[e, hc*128+p, j]) as [2, HK, KC, 128, 128]
    cw = np.asarray(conv_w, dtype=np.float32).reshape(2, HK, 128, KC)
    dgh = np.zeros((2, HK, KC, 128, 128), dtype=np.float32)
    pp = np.arange(128)
    dgh[:, :, :, pp, pp] = cw.transpose(0, 1, 3, 2)
    dgh = _bf16(dgh)

    in_maps = []
    for i in range(NCORES):
        lo = i * T
        if i % 2 == 0:
            halo = np.zeros((KC - 1, H), dtype=np.float32)
        else:
            halo = xflat[lo - (KC - 1) : lo]
        xh = np.concatenate([halo, xflat[lo : lo + T]], axis=0)  # [T+3, H]
        xf = np.ascontiguousarray(_bf16(xh).T)                   # [H, T+3]
        idxp = np.ascontiguousarray(
            idxflat[lo : lo + T].reshape(T // 128, 128, KTOP).transpose(1, 0, 2)
        ).astype(np.float32)
        nwp = np.ascontiguousarray(
            nwflat[lo : lo + T].reshape(T // 128, 128, KTOP).transpose(1, 0, 2)
        )
        in_maps.append(
            {"xf": xf, "wg": wg, "wu": wu, "wd": wd, "dgh": dgh,
             "idxp": idxp, "nwp": nwp}
        )
    return in_maps


def assemble(results):
    out = np.concatenate(
        [np.asarray(r["out"], dtype=np.float32) for r in results], axis=0
    )
    return out.reshape(B, S, H)


def kernel(x, top_k_indices, norm_weights, mlp_gate, mlp_up, mlp_down, conv_w):
    in_maps = build_in_maps(
        x, top_k_indices, norm_weights, mlp_gate, mlp_up, mlp_down, conv_w
    )
    nc = build_nc()
    res = run_bass_kernel_spmd(nc, in_maps, core_ids=list(range(NCORES)))
    return assemble(res.results)
